# revision 4
# baseline (speedup 1.0000x reference)
"""Correlation-volume kernel for trn2 (8 NeuronCores, batch-parallel).

out[n, (i,j), h, w] = sum_z imgA[n,z,h,w] * imgB[n,z,h+(j-4),w+(i-4)]
(zero padding outside the image; verified equivalent to the bilinear
reference to ~1e-5 relative).

Device strategy (per core, one batch element):
  - inputs cast to bf16 on host; B zero-padded to 168x168 on host.
  - 200 stationary blocks of 8x16=128 A-pixels; for each, one TensorE
    matmul contracts z=128 against a 16x24=384-column B window, giving a
    PSUM "gram" [128 pixels x 384 window positions] that contains all 81
    displacement dot-products per pixel on a diagonal band.
  - DVE/ACT copy PSUM->SBUF (cast bf16), SP DMA spills the raw grams to
    DRAM. The diagonal band extraction (gather) happens on host during
    the unshard, where strided views make it cheap.
"""

import numpy as np
import ml_dtypes
from numpy.lib.stride_tricks import as_strided

import concourse.bass as bass
import concourse.mybir as mybir
from concourse.bass_utils import run_bass_kernel_spmd

BF16 = mybir.dt.bfloat16
F32 = mybir.dt.float32

Z = 128
H = W = 160
PAD = 4
R = 9                      # displacements per axis
BH, BW = 8, 16             # stationary block (BH*BW == 128)
NBH, NBW = H // BH, W // BW
NB = NBH * NBW             # 200 blocks
MH, MW = BH + 2 * PAD, BW + 2 * PAD   # 16 x 24 moving window
MOV = MH * MW              # 384 columns per matmul
HP, WP = H + 2 * PAD, W + 2 * PAD     # 168 x 168 padded B
NPS = 8                    # PSUM banks in rotation
GRP = 16                   # blocks per spill group
NG = (NB + GRP - 1) // GRP # 13 groups (12x16 + 1x8)
STRIP = 16                 # input load strip (rows)

NP_BF16 = ml_dtypes.bfloat16


def _load_plan():
    """Interleaved input-load DMA order: B strip 0, A strip 0, B1, A1, ...

    B has ceil(168/16)=11 strips, A has 10. Returns list of ("a"|"b", j).
    """
    plan = []
    nb_strips = (HP + STRIP - 1) // STRIP
    na_strips = H // STRIP
    for j in range(nb_strips):
        plan.append(("b", j))
        if j < na_strips:
            plan.append(("a", j))
    return plan


def _load_threshold(bh, plan):
    """s_load value guaranteeing A rows [8bh,8bh+8) and B rows [8bh,8bh+16)."""
    need_b_row = 8 * bh + MH - 1         # last padded-B row needed
    need_a_row = 8 * bh + BH - 1
    jb = need_b_row // STRIP
    ja = need_a_row // STRIP
    last = 0
    for idx, (kind, j) in enumerate(plan):
        if (kind == "b" and j <= jb) or (kind == "a" and j <= ja):
            last = idx
    return 16 * (last + 1)


def build_nc():
    nc = bass.Bass()
    a = nc.declare_dram_parameter("a", [Z, H * W], BF16, isOutput=False)
    bp = nc.declare_dram_parameter("bp", [Z, HP * WP], BF16, isOutput=False)
    g = nc.declare_dram_parameter("g", [Z, NB * MOV], BF16, isOutput=True)

    plan = _load_plan()

    with (
        nc.sbuf_tensor([Z, H * W], BF16) as a_sb,
        nc.sbuf_tensor([Z, HP * WP], BF16) as b_sb,
        nc.sbuf_tensor([Z, GRP * MOV], BF16) as stage0,
        nc.sbuf_tensor([Z, GRP * MOV], BF16) as stage1,
        nc.psum_tensor([Z, MOV], F32) as ps0,
        nc.psum_tensor([Z, MOV], F32) as ps1,
        nc.psum_tensor([Z, MOV], F32) as ps2,
        nc.psum_tensor([Z, MOV], F32) as ps3,
        nc.psum_tensor([Z, MOV], F32) as ps4,
        nc.psum_tensor([Z, MOV], F32) as ps5,
        nc.psum_tensor([Z, MOV], F32) as ps6,
        nc.psum_tensor([Z, MOV], F32) as ps7,
        nc.semaphore("s_load") as s_load,
        nc.semaphore("s_mm") as s_mm,
        nc.semaphore("s_cpv") as s_cpv,
        nc.semaphore("s_cpa") as s_cpa,
        nc.semaphore("s_spill") as s_spill,
        nc.Block() as block,
    ):
        psum = [ps0, ps1, ps2, ps3, ps4, ps5, ps6, ps7]
        stage = [stage0, stage1]
        b3 = b_sb[:].rearrange("p (h w) -> p h w", h=HP)
        b3d = bp[:].rearrange("p (h w) -> p h w", h=HP)

        @block.sync
        def _(sync):
            # input strip loads (no waits -> issue immediately, FIFO)
            # "a" is block-major on host: strip of 16 image rows = 2 block
            # rows = contiguous columns.
            for kind, j in plan:
                if kind == "a":
                    c0, c1 = j * STRIP * W, (j + 1) * STRIP * W
                    sync.dma_start(
                        out=a_sb[:, c0:c1], in_=a[:, c0:c1]
                    ).then_inc(s_load, 16)
                else:
                    r0, r1 = j * STRIP, min((j + 1) * STRIP, HP)
                    sync.dma_start(
                        out=b3[:, r0:r1, :], in_=b3d[:, r0:r1, :]
                    ).then_inc(s_load, 16)
            # gram spills
            for gi in range(NG):
                nblk = min(GRP, NB - gi * GRP)
                ndone = (gi * GRP + nblk) // 2
                sync.wait_ge(s_cpv, ndone)
                sync.wait_ge(s_cpa, ndone)
                sync.dma_start(
                    out=g[:, gi * GRP * MOV:(gi * GRP + nblk) * MOV],
                    in_=stage[gi % 2][:, :nblk * MOV],
                ).then_inc(s_spill, 16)
            sync.wait_ge(s_spill, 16 * NG)

        @block.tensor
        def _(tensor):
            for b in range(NB):
                bh, bw = divmod(b, NBW)
                if bw == 0:
                    tensor.wait_ge(s_load, _load_threshold(bh, plan))
                if b >= NPS:
                    pb = b - NPS
                    if pb % 2 == 0:
                        tensor.wait_ge(s_cpv, pb // 2 + 1)
                    else:
                        tensor.wait_ge(s_cpa, pb // 2 + 1)
                h0, w0 = bh * BH, bw * BW
                nc.tensor.matmul(
                    psum[b % NPS][:, :],
                    a_sb[:, b * 128:(b + 1) * 128],
                    b3[:, h0:h0 + MH, w0:w0 + MW],
                    start=True,
                    stop=True,
                ).then_inc(s_mm, 1)

        @block.vector
        def _(vector):
            for b in range(0, NB, 2):
                gi, sl = b // GRP, b % GRP
                if sl == 0 and gi >= 2:
                    vector.wait_ge(s_spill, 16 * (gi - 1))
                vector.wait_ge(s_mm, b + 1)
                nc.vector.tensor_copy(
                    stage[gi % 2][:, sl * MOV:(sl + 1) * MOV],
                    psum[b % NPS][:, :],
                ).then_inc(s_cpv, 1)

        @block.scalar
        def _(scalar):
            for b in range(1, NB, 2):
                gi, sl = b // GRP, b % GRP
                if sl == 1 and gi >= 2:
                    scalar.wait_ge(s_spill, 16 * (gi - 1))
                scalar.wait_ge(s_mm, b + 1)
                nc.scalar.copy(
                    stage[gi % 2][:, sl * MOV:(sl + 1) * MOV],
                    psum[b % NPS][:, :],
                ).then_inc(s_cpa, 1)

    return nc


def prep_core(An, Bn):
    """An, Bn: [Z,H,W] float32 -> per-core input map (bf16, B padded).

    "a" is laid out block-major: [z, bh, bw, h_l, w_l] so each stationary
    block's 128 pixels are contiguous (walrus: weights AP must be 1-D free).
    """
    a = (
        An.reshape(Z, NBH, BH, NBW, BW)
        .transpose(0, 1, 3, 2, 4)
        .reshape(Z, H * W)
        .astype(NP_BF16)
    )
    bpad = np.zeros((Z, HP, WP), NP_BF16)
    bpad[:, PAD:PAD + H, PAD:PAD + W] = Bn
    return {"a": np.ascontiguousarray(a), "bp": bpad.reshape(Z, HP * WP)}


def extract_core(gres):
    """gres: [Z, NB*MOV] bf16 gram spill -> [81,H,W] float32 output."""
    G6 = np.ascontiguousarray(gres).reshape(BH, BW, NBH, NBW, MH, MW)
    st = G6.strides
    out = np.empty((R * R, H, W), np.float32)
    for dx in range(-PAD, PAD + 1):
        for dy in range(-PAD, PAD + 1):
            k = (dx + PAD) * R + (dy + PAD)
            base = G6[:, :, :, :, PAD + dy, PAD + dx]
            V = as_strided(
                base,
                shape=(BH, BW, NBH, NBW),
                strides=(st[0] + st[4], st[1] + st[5], st[2], st[3]),
            )
            out[k] = V.transpose(2, 0, 3, 1).astype(np.float32).reshape(H, W)
    return out


_NC_CACHE = {}


def get_nc():
    if "nc" not in _NC_CACHE:
        _NC_CACHE["nc"] = build_nc()
    return _NC_CACHE["nc"]


def kernel(imgA, imgB):
    imgA = np.asarray(imgA)
    imgB = np.asarray(imgB)
    N = imgA.shape[0]
    in_maps = [prep_core(imgA[n], imgB[n]) for n in range(N)]
    res = run_bass_kernel_spmd(get_nc(), in_maps, list(range(N)))
    return np.stack([extract_core(res.results[n]["g"]) for n in range(N)])
